# revision 25
# baseline (speedup 1.0000x reference)
"""Multi-head self-attention (B=4, C=256, H=W=48, NH=8) on 8 TRN2 NeuronCores.

Sharding: 8 shards = 4 batches x 2 query-halves (no collectives).

v2 design (vs baseline at 252us):
  - exp split across ScalarE (true exp, ~55%) and VectorE (Schraudolph
    bit-trick exp: one tensor_scalar f32->int16 rint-convert bitcast over a
    bf16 tile; max rel err ~3.3%, washes out in softmax + residual).
  - chunk-major attention: all 4 head-pairs advance together per q-chunk,
    so consecutive score matmuls cycle PE row groups 0/32/64/96 (4-way
    tile_position packing) and the PE stream stays dense (HAM warm).
  - normalization: ScalarE exits av [0:97] PSUM->SBUF; denominators DMA to
    DRAM, ONE batched [128,16] DVE reciprocal per chunk, recip rows DMA
    back broadcast (stride-0 partition from DRAM), one fused [97,2,256]
    DVE multiply writes f32r; SBUF->SBUF DMAs shift rows into att layout.
    (replaces forty 1.6us single-partition RECIPROCALs)
  - host folds: V-bias into bo' = bo + Wo bv; softmax SCALE into Wq/bq;
    x/weights pre-cast to bf16 on host (no on-chip casts).
  - out-projection: fused (ps + bo')+x residual in one scalar_tensor_tensor.
"""

import numpy as np
import ml_dtypes

import concourse.bass as bass
import concourse.mybir as mybir
import concourse.tile as tile
from concourse.vector_clock import ScopedClock
from concourse.bass_utils import run_bass_kernel_spmd

B, C, HH, WW = 4, 256, 48, 48
S = HH * WW            # 2304
NH, HD = 8, 32
SCALE = HD ** -0.5
SQ = S // 2            # 1152 queries per core
CT = C // 128          # 2 channel tiles
NTT = S // 128         # 18 t-tiles
TG = 3                 # t-tiles per exp group
NTG = NTT // TG        # 6

F32 = mybir.dt.float32
F32R = mybir.dt.float32r
BF16 = mybir.dt.bfloat16
I16 = mybir.dt.int16
AF = mybir.ActivationFunctionType
ALU = mybir.AluOpType

N_CORES = 8

# attention/output q-chunks: 4x256 + 1x128
JCH = [(0, 256), (256, 256), (512, 256), (768, 256), (1024, 128)]
# projection chunks
KCH = [(0, 512), (512, 512), (1024, 512), (1536, 512), (2048, 256)]
QCH = [(0, 512), (512, 512), (1024, 128)]

# Schraudolph exp constants (scores arrive pre-scaled by SCALE via Wq/bq):
# bf16_bits(exp(x)) ~= rint(x * 128/ln2 + (127*128 - c))
EXP_A = 128.0 / float(np.log(2.0))
EXP_B = 127.0 * 128.0 - 0.0430357 * 128.0

# fraction of exp units sent to ScalarE (rest go to VectorE Schraudolph)
A_SHARE = 0.51

DEBUG = False


class _TileContextP(tile.TileContext):
    """TileContext adapted to a walrus that allows 1 sem wait/instruction.

    After Tile scheduling, every instruction carrying N>1 sem waits is
    rewritten to keep its last wait; the other N-1 waits move onto
    fresh single-wait nops inserted just before it on the same engine
    (engines execute their stream in order, so blocking at the nop is
    equivalent). The kernel-tail drain is built the same way.
    """

    def _split_multi_waits(self):
        nc = self.nc
        for fn in nc.m.functions:
            for bb in fn.blocks:
                new_insts = []
                for inst in bb.instructions:
                    si = inst.sync_info
                    if si is not None and len(si.on_wait) > 1:
                        waits = list(si.on_wait)
                        for w in waits[:-1]:
                            nop = mybir.InstNoOp(
                                name=nc.get_next_instruction_name(),
                                engine=inst.engine,
                                ins=[], outs=[],
                                sync_info=mybir.SyncInfo(on_wait=[w], on_update=[]),
                                bass_nofuse=True,
                            )
                            nc.register_instruction(nop, overwrite=True)
                            new_insts.append(nop)
                        inst.sync_info = mybir.SyncInfo(
                            on_wait=[waits[-1]], on_update=list(si.on_update)
                        )
                    new_insts.append(inst)
                bb.instructions = new_insts

    def _drain_and_barrier(self, tick_clock, wait_clock):
        carrier = self.nc.sync.nop(nofuse=True)
        wait_clock.add_sem_waits(
            carrier.ins, ScopedClock({None: tick_clock.global_clock})
        )
        self.nc.sync.drain()
        self.nc.all_engine_barrier()
        assert self.sems is not None
        popped = self.nc._tile_sem_poison_stack.pop()
        assert popped is self._sem_poison
        self.nc.clear_and_free_semaphores(list(self.sems.allocated().values()))
        self.nc.all_engine_barrier()
        self._split_multi_waits()


def _build_nc():
    nc = bass.Bass()

    xf_d = nc.dram_tensor("xfb", [C, S], BF16, kind="ExternalInput")
    xqb_d = nc.dram_tensor("xqb", [C, SQ], BF16, kind="ExternalInput")
    xq_d = nc.dram_tensor("xq", [C, SQ], F32, kind="ExternalInput")
    wqt_d = nc.dram_tensor("wqtb", [C, C], BF16, kind="ExternalInput")
    wkt_d = nc.dram_tensor("wktb", [C, C], BF16, kind="ExternalInput")
    wvt_d = nc.dram_tensor("wvtb", [C, C], BF16, kind="ExternalInput")
    wot_d = nc.dram_tensor("wot", [C, C], F32, kind="ExternalInput")
    bqp_d = nc.dram_tensor("bqp", [128, CT], F32, kind="ExternalInput")
    bop_d = nc.dram_tensor("bop", [128, CT], F32, kind="ExternalInput")
    out_d = nc.dram_tensor("out", [C, SQ], F32, kind="ExternalOutput")

    dbg = {}
    if DEBUG:
        dbg["k0"] = nc.dram_tensor("dbg_k0", [128, S], BF16, kind="ExternalOutput")
        dbg["q0"] = nc.dram_tensor("dbg_q0", [128, SQ], BF16, kind="ExternalOutput")
        dbg["vt"] = nc.dram_tensor("dbg_vt", [128, NTT, NH, HD + 1], BF16,
                                   kind="ExternalOutput")
        dbg["ex0"] = nc.dram_tensor("dbg_ex0", [128, 2 * TG, 256], BF16,
                                    kind="ExternalOutput")
        dbg["ex1"] = nc.dram_tensor("dbg_ex1", [128, 2 * TG, 256], BF16,
                                    kind="ExternalOutput")
        dbg["avs0"] = nc.dram_tensor("dbg_avs0", [97, 256], F32,
                                     kind="ExternalOutput")
        dbg["den"] = nc.dram_tensor("dbg_den", [NH, SQ], F32, kind="ExternalOutput")
        dbg["rec"] = nc.dram_tensor("dbg_rec", [NH, SQ], F32, kind="ExternalOutput")
        dbg["bc0"] = nc.dram_tensor("dbg_bc0", [128, 256], F32,
                                    kind="ExternalOutput")
        dbg["att0"] = nc.dram_tensor("dbg_att0", [128, CT, 256], F32,
                                     kind="ExternalOutput")

    with _TileContextP(nc) as tc:
        with (
            tc.tile_pool(name="singles", bufs=1) as singles,
            tc.tile_pool(name="sbig", bufs=1) as sbig,
            tc.tile_pool(name="expsp", bufs=8) as expsp,
            tc.tile_pool(name="avsbp", bufs=4) as avsbp,
            tc.tile_pool(name="bcp", bufs=4) as bcp,
            tc.tile_pool(name="dsbp", bufs=2) as dsbp,
            tc.tile_pool(name="outp", bufs=4) as outp,
            tc.tile_pool(name="drp", bufs=4, space="DRAM") as drp,
        ):
            # ---- static loads (host pre-cast bf16; no on-chip casts) ----
            w_bf = {}
            for nm, d in (("wqt", wqt_d), ("wkt", wkt_d), ("wvt", wvt_d)):
                ld = singles.tile([128, CT, C], BF16, tag=f"{nm}_ld")
                nc.sync.dma_start(out=ld, in_=d.rearrange("(t p) o -> p t o", p=128))
                w_bf[nm] = ld
            wqt_sb, wkt_sb, wvt_sb = w_bf["wqt"], w_bf["wkt"], w_bf["wvt"]

            bqp_sb = singles.tile([128, CT], F32)
            nc.sync.dma_start(out=bqp_sb, in_=bqp_d[:, :])
            bop_sb = singles.tile([128, CT], F32)
            nc.sync.dma_start(out=bop_sb, in_=bop_d[:, :])

            x_bf = sbig.tile([128, CT, S], BF16, name="x_bf")
            nc.scalar.dma_start(out=x_bf, in_=xf_d.rearrange("(t p) s -> p t s", p=128))
            xq_bf = sbig.tile([128, CT, SQ], BF16, name="xq_bf")
            nc.sync.dma_start(out=xq_bf, in_=xqb_d.rearrange("(t p) s -> p t s", p=128))
            xq_ld = sbig.tile([128, CT, SQ], F32, name="xq_ld")
            nc.gpsimd.dma_start(out=xq_ld, in_=xq_d.rearrange("(t p) s -> p t s", p=128))

            wot_ld = singles.tile([128, CT, C], F32, tag="wot_ld")
            nc.sync.dma_start(out=wot_ld, in_=wot_d.rearrange("(t p) o -> p t o", p=128))
            wot_sb = singles.tile([128, CT, C], F32R, tag="wot_rb")
            nc.vector.tensor_copy(out=wot_sb, in_=wot_ld)

            k_t = [sbig.tile([128, S], BF16, name=f"k{t}") for t in range(CT)]
            q_t = [sbig.tile([128, SQ], BF16, name=f"q{t}") for t in range(CT)]
            # V^T with ones denominator column: [t, st, head, HD+1]
            vt = sbig.tile([128, NTT, NH, HD + 1], BF16, name="vt")
            nc.vector.memset(vt, 1.0)
            att_j = [sbig.tile([128, CT, ln], F32R, name=f"att{i}")
                     for i, (j0, ln) in enumerate(JCH)]

            den_d = drp.tile([NH, SQ], F32, tag="den")
            rec_d = drp.tile([NH, SQ], F32, tag="rec")

            # ---- phase A: projections (all-bf16 matmuls) ----------------
            # copies out of PSUM ride ScalarE (ACT), freeing VectorE for exp
            def k_proj(ot, psA):
                for (j0, ln) in KCH:
                    ps = psA.tile([128, 512], F32, tag="proj", name=f"kp{ot}{j0}")
                    for kt in range(CT):
                        nc.tensor.matmul(
                            ps[:, 0:ln],
                            lhsT=wkt_sb[:, kt, ot * 128:(ot + 1) * 128],
                            rhs=x_bf[:, kt, j0:j0 + ln],
                            start=(kt == 0), stop=(kt == CT - 1),
                        )
                    nc.scalar.activation(
                        out=k_t[ot][:, j0:j0 + ln], in_=ps[:, 0:ln], func=AF.Copy,
                    )

            def q_proj(ot, psA):
                for (j0, ln) in QCH:
                    ps = psA.tile([128, 512], F32, tag="proj", name=f"qp{ot}{j0}")
                    for kt in range(CT):
                        nc.tensor.matmul(
                            ps[:, 0:ln],
                            lhsT=wqt_sb[:, kt, ot * 128:(ot + 1) * 128],
                            rhs=xq_bf[:, kt, j0:j0 + ln],
                            start=(kt == 0), stop=(kt == CT - 1),
                        )
                    nc.scalar.activation(
                        out=q_t[ot][:, j0:j0 + ln], in_=ps[:, 0:ln],
                        func=AF.Identity, bias=bqp_sb[:, ot:ot + 1],
                    )

            def v_proj(sv, psA):
                ps = psA.tile([128, 2, C], F32, tag="proj", name=f"vp{sv}")
                for sti in range(2):
                    st = 2 * sv + sti
                    for kt in range(CT):
                        nc.tensor.matmul(
                            ps[:, sti, :],
                            lhsT=x_bf[:, kt, st * 128:(st + 1) * 128],
                            rhs=wvt_sb[:, kt, :],
                            start=(kt == 0), stop=(kt == CT - 1),
                        )
                nc.scalar.activation(
                    out=vt[:, 2 * sv:2 * sv + 2, :, 0:HD],
                    in_=ps.rearrange("p s (h d) -> p s h d", h=NH),
                    func=AF.Copy,
                )

            with tc.tile_pool(name="psA", bufs=4, space="PSUM") as psA:
                k_proj(0, psA)
                q_proj(0, psA)
                k_proj(1, psA)
                q_proj(1, psA)
                for sv in range(NTT // 2):
                    v_proj(sv, psA)

            if DEBUG:
                nc.sync.dma_start(out=dbg["k0"][:, :], in_=k_t[0])
                nc.sync.dma_start(out=dbg["q0"][:, :], in_=q_t[0])
                nc.sync.dma_start(out=dbg["vt"][:, :, :, :], in_=vt)

            # ---- phase B: chunk-major attention -------------------------
            # head h: channel tile ct=h//4, offset co=32*(h%4).
            # pair hp: heads (2hp, 2hp+1). av tile side a: pairs (2a, 2a+1),
            # PSUM layout [128, pair-slot s, 256]: rows 64*hi + [0:32] vals,
            # row 64*hi+32 denominator.
            exp_acc = [0.5]

            def emit_exp(sc, ex, ln):
                exp_acc[0] += A_SHARE
                if exp_acc[0] >= 1.0:
                    exp_acc[0] -= 1.0
                    nc.scalar.activation(
                        out=ex[:, :, 0:ln], in_=sc[:, :, 0:ln], func=AF.Exp,
                    )
                else:
                    nc.vector.tensor_scalar(
                        out=ex[:, :, 0:ln].bitcast(I16), in0=sc[:, :, 0:ln],
                        scalar1=EXP_A, scalar2=EXP_B,
                        op0=ALU.mult, op1=ALU.add,
                    )

            def attention_chunk(jidx, scp, avp):
                j0, ln = JCH[jidx]
                js = slice(j0, j0 + ln)
                # Each pair gets a FULL PSUM bank ([128, 512] f32): two
                # concurrently-open accumulations may not share a bank at
                # the same partitions (start=True clears the whole bank row
                # for the partitions written). The two hi regions inside a
                # pair are partition-disjoint (rows 0-32 / 64-96). The two
                # pair-duos run sequentially so only 2 av banks are live.
                avs_t = []
                for a in range(2):              # a: pair-duo = channel tile
                    av = [avp.tile([128, 512], F32, tag="av",
                                   name=f"av{jidx}{a}{hpi}")
                          for hpi in range(2)]
                    for g in range(NTG):
                        sc = [scp.tile([128, 2 * TG, 256], F32, tag="sc",
                                       name=f"sc{jidx}{g}{a}{hpi}")
                              for hpi in range(2)]
                        for tt in range(TG):
                            t0 = (g * TG + tt) * 128
                            for hpi in range(2):    # pair within duo
                                for hi in range(2):
                                    co = 32 * (2 * hpi + hi)
                                    nc.tensor.matmul(
                                        sc[hpi][:, hi * TG + tt, 0:ln],
                                        lhsT=k_t[a][co:co + HD, t0:t0 + 128],
                                        rhs=q_t[a][co:co + HD, js],
                                        start=True, stop=True,
                                        tile_position=(co, 0),
                                    )
                        ex = [expsp.tile([128, 2 * TG, 256], BF16, tag="ex",
                                         name=f"ex{jidx}{g}{a}{hpi}")
                              for hpi in range(2)]
                        for hpi in range(2):
                            emit_exp(sc[hpi], ex[hpi], ln)
                        if DEBUG and jidx == 0 and g == 0 and a == 0:
                            for hpi in range(2):
                                nc.sync.dma_start(out=dbg[f"ex{hpi}"][:, :, :],
                                                  in_=ex[hpi])
                        for tt in range(TG):
                            st = g * TG + tt
                            first = (g == 0 and tt == 0)
                            last = (g == NTG - 1 and tt == TG - 1)
                            for hpi in range(2):
                                for hi in range(2):
                                    h = 4 * a + 2 * hpi + hi
                                    nc.tensor.matmul(
                                        av[hpi][64 * hi:64 * hi + HD + 1, 0:ln],
                                        lhsT=vt[:, st, h, :],
                                        rhs=ex[hpi][:, hi * TG + tt, 0:ln],
                                        start=first, stop=last,
                                        tile_position=(0, 64 * hi),
                                        skip_group_check=True,
                                    )
                    # exit av from PSUM (ScalarE); denominators -> DRAM
                    for hpi in range(2):
                        avs = avsbp.tile([97, 256], F32, tag="avs",
                                         name=f"avs{jidx}{a}{hpi}")
                        avs_t.append(avs)
                        nc.scalar.activation(
                            out=avs[:, 0:ln], in_=av[hpi][0:97, 0:ln],
                            func=AF.Copy,
                        )
                        for hi in range(2):
                            h = 4 * a + 2 * hpi + hi
                            nc.sync.dma_start(
                                out=den_d[h:h + 1, j0:j0 + ln],
                                in_=avs[64 * hi + 32:64 * hi + 33, 0:ln],
                            )
                # batched reciprocal of all 8 denominators for this chunk
                nh = ln // 128
                dsb = dsbp.tile([128, NH, 4], F32, tag="dsb", name=f"dsb{jidx}")
                rcb = dsbp.tile([128, NH, 4], F32, tag="rcb", name=f"rcb{jidx}")
                for f in range(nh):
                    qf = j0 + 128 * f
                    nc.gpsimd.dma_start(
                        out=dsb[:, :, f:f + 1],
                        in_=den_d[:, qf:qf + 128].rearrange("h (p o) -> p h o", o=1),
                    )
                nc.vector.reciprocal(rcb[:, :, 0:nh], dsb[:, :, 0:nh])
                for f in range(nh):
                    qf = j0 + 128 * f
                    nc.gpsimd.dma_start(
                        out=rec_d[:, qf:qf + 128].rearrange("h (p o) -> p h o", o=1),
                        in_=rcb[:, :, f:f + 1],
                    )
                # broadcast recips + fused normalize multiply -> f32r
                for p in range(4):              # pair index; heads 2p, 2p+1
                    a, hpi = p // 2, p % 2
                    bc = bcp.tile([128, 256], F32, tag="bc", name=f"bc{jidx}{p}")
                    # rows 0-63 <- rec[2p], rows 64-127 <- rec[2p+1]
                    for hi in range(2):
                        rr = rec_d[2 * p + hi:2 * p + hi + 1, j0:j0 + ln]
                        nc.scalar.dma_start(
                            out=bc[64 * hi:64 * hi + 64, 0:ln],
                            in_=bass.AP(
                                tensor=rr.tensor, offset=rr.offset,
                                ap=[[0, 64]] + [list(x) for x in rr.ap[1:]],
                            ),
                        )
                    if DEBUG and jidx == 0 and p == 0:
                        nc.sync.dma_start(out=dbg["avs0"][:, :], in_=avs_t[0])
                        nc.sync.dma_start(out=dbg["bc0"][:, :], in_=bc)
                    nrm = avsbp.tile([97, 256], F32R, tag="nrm",
                                     name=f"nrm{jidx}{p}")
                    nc.vector.tensor_tensor(
                        out=nrm[:, 0:ln], in0=avs_t[p][:, 0:ln],
                        in1=bc[0:97, 0:ln], op=ALU.mult,
                    )
                    # att layout: head h -> ct h//4, rows 32*(h%4)
                    r0 = 64 * (p % 2)
                    nc.sync.dma_start(
                        out=att_j[jidx][r0:r0 + 32, a, 0:ln],
                        in_=nrm[0:32, 0:ln],
                    )
                    nc.sync.dma_start(
                        out=att_j[jidx][r0 + 32:r0 + 64, a, 0:ln],
                        in_=nrm[64:96, 0:ln],
                    )

            with (
                tc.tile_pool(name="scp", bufs=2, space="PSUM") as scp,
                tc.tile_pool(name="avp", bufs=2, space="PSUM") as avp,
            ):
                for jidx in range(len(JCH)):
                    attention_chunk(jidx, scp, avp)

            if DEBUG:
                nc.sync.dma_start(out=dbg["den"][:, :], in_=den_d[:, :])
                nc.sync.dma_start(out=dbg["rec"][:, :], in_=rec_d[:, :])
                nc.sync.dma_start(out=dbg["att0"][:, :, :],
                                  in_=att_j[0][:, :, :].bitcast(F32))

            # ---- phase C: output projection + residual ------------------
            out_r = out_d.rearrange("(t p) q -> p t q", p=128)

            def out_proj_chunk(jidx, psC):
                j0, ln = JCH[jidx]
                js = slice(j0, j0 + ln)
                for ot in range(CT):
                    # full PSUM bank per accumulator: co-tenant accumulation
                    # groups in one bank wipe each other on start=True
                    ps = psC.tile([128, 512], F32, tag="cps", name=f"cps{jidx}{ot}")
                    for kt in range(CT):
                        nc.tensor.matmul(
                            ps[:, 0:ln],
                            lhsT=wot_sb[:, kt, ot * 128:(ot + 1) * 128],
                            rhs=att_j[jidx][:, kt, 0:ln],
                            start=(kt == 0), stop=(kt == CT - 1),
                        )
                    ob = outp.tile([128, 256], F32, tag="ob", name=f"ob{jidx}{ot}")
                    nc.vector.scalar_tensor_tensor(
                        out=ob[:, 0:ln], in0=ps[:, 0:ln],
                        scalar=bop_sb[:, ot:ot + 1], in1=xq_ld[:, ot, js],
                        op0=ALU.add, op1=ALU.add,
                    )
                    nc.sync.dma_start(out=out_r[:, ot, js], in_=ob[:, 0:ln])

            with tc.tile_pool(name="psC", bufs=4, space="PSUM") as psC:
                for jidx in range(len(JCH)):
                    out_proj_chunk(jidx, psC)
    return nc


_NC = None
LAST_RESULTS = None
TRACE = False


def _get_nc():
    global _NC
    if _NC is None:
        _NC = _build_nc()
    return _NC


def kernel(x, Wq, bq, Wk, bk, Wv, bv, Wo, bo):
    global LAST_RESULTS
    bf = ml_dtypes.bfloat16
    x = np.ascontiguousarray(np.asarray(x, dtype=np.float32).reshape(B, C, S))
    x_bf = x.astype(bf)
    Wq = np.asarray(Wq, dtype=np.float32)
    Wo = np.asarray(Wo, dtype=np.float32)
    bv = np.asarray(bv, dtype=np.float32)
    wqt = np.ascontiguousarray((Wq * SCALE).T.astype(bf))
    wkt = np.ascontiguousarray(np.asarray(Wk, dtype=np.float32).T.astype(bf))
    wvt = np.ascontiguousarray(np.asarray(Wv, dtype=np.float32).T.astype(bf))
    wot = np.ascontiguousarray(Wo.T)
    bqp = np.ascontiguousarray(
        (np.asarray(bq, dtype=np.float32) * SCALE).reshape(CT, 128).T)
    bo2 = np.asarray(bo, dtype=np.float32) + Wo @ bv
    bop = np.ascontiguousarray(bo2.reshape(CT, 128).T)

    in_maps = []
    for core in range(N_CORES):
        b, half = divmod(core, 2)
        qlo = half * SQ
        in_maps.append({
            "xfb": x_bf[b],
            "xqb": np.ascontiguousarray(x_bf[b][:, qlo:qlo + SQ]),
            "xq": np.ascontiguousarray(x[b][:, qlo:qlo + SQ]),
            "wqtb": wqt, "wktb": wkt, "wvtb": wvt, "wot": wot,
            "bqp": bqp, "bop": bop,
        })

    res = run_bass_kernel_spmd(_get_nc(), in_maps, list(range(N_CORES)), trace=TRACE)
    LAST_RESULTS = res

    out = np.empty((B, C, S), dtype=np.float32)
    for core in range(N_CORES):
        b, half = divmod(core, 2)
        qlo = half * SQ
        out[b][:, qlo:qlo + SQ] = res.results[core]["out"]
    return out.reshape(B, C, HH, WW)


# revision 27
# speedup vs baseline: 1.0025x; 1.0025x over previous
"""Multi-head self-attention (B=4, C=256, H=W=48, NH=8) on 8 TRN2 NeuronCores.

Sharding: 8 shards = 4 batches x 2 query-halves (no collectives).

v2 design (vs baseline at 252us):
  - exp split across ScalarE (true exp, ~55%) and VectorE (Schraudolph
    bit-trick exp: one tensor_scalar f32->int16 rint-convert bitcast over a
    bf16 tile; max rel err ~3.3%, washes out in softmax + residual).
  - chunk-major attention: all 4 head-pairs advance together per q-chunk,
    so consecutive score matmuls cycle PE row groups 0/32/64/96 (4-way
    tile_position packing) and the PE stream stays dense (HAM warm).
  - normalization: ScalarE exits av [0:97] PSUM->SBUF; denominators DMA to
    DRAM, ONE batched [128,16] DVE reciprocal per chunk, recip rows DMA
    back broadcast (stride-0 partition from DRAM), one fused [97,2,256]
    DVE multiply writes f32r; SBUF->SBUF DMAs shift rows into att layout.
    (replaces forty 1.6us single-partition RECIPROCALs)
  - host folds: V-bias into bo' = bo + Wo bv; softmax SCALE into Wq/bq;
    x/weights pre-cast to bf16 on host (no on-chip casts).
  - out-projection: fused (ps + bo')+x residual in one scalar_tensor_tensor.
"""

import numpy as np
import ml_dtypes

import concourse.bass as bass
import concourse.mybir as mybir
import concourse.tile as tile
from concourse.vector_clock import ScopedClock
from concourse.bass_utils import run_bass_kernel_spmd

B, C, HH, WW = 4, 256, 48, 48
S = HH * WW            # 2304
NH, HD = 8, 32
SCALE = HD ** -0.5
SQ = S // 2            # 1152 queries per core
CT = C // 128          # 2 channel tiles
NTT = S // 128         # 18 t-tiles
TG = 3                 # t-tiles per exp group
NTG = NTT // TG        # 6

F32 = mybir.dt.float32
F32R = mybir.dt.float32r
BF16 = mybir.dt.bfloat16
I16 = mybir.dt.int16
AF = mybir.ActivationFunctionType
ALU = mybir.AluOpType

N_CORES = 8

# attention/output q-chunks: 4x256 + 1x128
JCH = [(0, 256), (256, 256), (512, 256), (768, 256), (1024, 128)]
# projection chunks
KCH = [(0, 512), (512, 512), (1024, 512), (1536, 512), (2048, 256)]
QCH = [(0, 512), (512, 512), (1024, 128)]

# Schraudolph exp constants (scores arrive pre-scaled by SCALE via Wq/bq):
# bf16_bits(exp(x)) ~= rint(x * 128/ln2 + (127*128 - c))
EXP_A = 128.0 / float(np.log(2.0))
EXP_B = 127.0 * 128.0 - 0.0430357 * 128.0

# fraction of exp units sent to ScalarE (rest go to VectorE Schraudolph)
A_SHARE = 0.51

DEBUG = False


class _TileContextP(tile.TileContext):
    """TileContext adapted to a walrus that allows 1 sem wait/instruction.

    After Tile scheduling, every instruction carrying N>1 sem waits is
    rewritten to keep its last wait; the other N-1 waits move onto
    fresh single-wait nops inserted just before it on the same engine
    (engines execute their stream in order, so blocking at the nop is
    equivalent). The kernel-tail drain is built the same way.
    """

    def _split_multi_waits(self):
        nc = self.nc
        for fn in nc.m.functions:
            for bb in fn.blocks:
                new_insts = []
                for inst in bb.instructions:
                    si = inst.sync_info
                    if si is not None and len(si.on_wait) > 1:
                        waits = list(si.on_wait)
                        for w in waits[:-1]:
                            nop = mybir.InstNoOp(
                                name=nc.get_next_instruction_name(),
                                engine=inst.engine,
                                ins=[], outs=[],
                                sync_info=mybir.SyncInfo(on_wait=[w], on_update=[]),
                                bass_nofuse=True,
                            )
                            nc.register_instruction(nop, overwrite=True)
                            new_insts.append(nop)
                        inst.sync_info = mybir.SyncInfo(
                            on_wait=[waits[-1]], on_update=list(si.on_update)
                        )
                    new_insts.append(inst)
                bb.instructions = new_insts

    def _drain_and_barrier(self, tick_clock, wait_clock):
        carrier = self.nc.sync.nop(nofuse=True)
        wait_clock.add_sem_waits(
            carrier.ins, ScopedClock({None: tick_clock.global_clock})
        )
        self.nc.sync.drain()
        self.nc.all_engine_barrier()
        assert self.sems is not None
        popped = self.nc._tile_sem_poison_stack.pop()
        assert popped is self._sem_poison
        self.nc.clear_and_free_semaphores(list(self.sems.allocated().values()))
        self.nc.all_engine_barrier()
        self._split_multi_waits()


def _build_nc():
    nc = bass.Bass()

    xf_d = nc.dram_tensor("xfb", [C, S], BF16, kind="ExternalInput")
    xqb_d = nc.dram_tensor("xqb", [C, SQ], BF16, kind="ExternalInput")
    xq_d = nc.dram_tensor("xq", [C, SQ], F32, kind="ExternalInput")
    wqt_d = nc.dram_tensor("wqtb", [C, C], BF16, kind="ExternalInput")
    wkt_d = nc.dram_tensor("wktb", [C, C], BF16, kind="ExternalInput")
    wvt_d = nc.dram_tensor("wvtb", [C, C], BF16, kind="ExternalInput")
    wot_d = nc.dram_tensor("wot", [C, C], F32, kind="ExternalInput")
    bqp_d = nc.dram_tensor("bqp", [128, CT], F32, kind="ExternalInput")
    bop_d = nc.dram_tensor("bop", [128, CT], F32, kind="ExternalInput")
    out_d = nc.dram_tensor("out", [C, SQ], F32, kind="ExternalOutput")

    dbg = {}
    if DEBUG:
        dbg["k0"] = nc.dram_tensor("dbg_k0", [128, S], BF16, kind="ExternalOutput")
        dbg["q0"] = nc.dram_tensor("dbg_q0", [128, SQ], BF16, kind="ExternalOutput")
        dbg["vt"] = nc.dram_tensor("dbg_vt", [128, NTT, NH, HD + 1], BF16,
                                   kind="ExternalOutput")
        dbg["ex0"] = nc.dram_tensor("dbg_ex0", [128, 2 * TG, 256], BF16,
                                    kind="ExternalOutput")
        dbg["ex1"] = nc.dram_tensor("dbg_ex1", [128, 2 * TG, 256], BF16,
                                    kind="ExternalOutput")
        dbg["avs0"] = nc.dram_tensor("dbg_avs0", [97, 256], F32,
                                     kind="ExternalOutput")
        dbg["den"] = nc.dram_tensor("dbg_den", [NH, SQ], F32, kind="ExternalOutput")
        dbg["rec"] = nc.dram_tensor("dbg_rec", [NH, SQ], F32, kind="ExternalOutput")
        dbg["bc0"] = nc.dram_tensor("dbg_bc0", [128, 256], F32,
                                    kind="ExternalOutput")
        dbg["att0"] = nc.dram_tensor("dbg_att0", [128, CT, 256], F32,
                                     kind="ExternalOutput")

    with _TileContextP(nc) as tc:
        with (
            tc.tile_pool(name="singles", bufs=1) as singles,
            tc.tile_pool(name="sbig", bufs=1) as sbig,
            tc.tile_pool(name="expsp", bufs=8) as expsp,
            tc.tile_pool(name="avsbp", bufs=4) as avsbp,
            tc.tile_pool(name="bcp", bufs=4) as bcp,
            tc.tile_pool(name="dsbp", bufs=2) as dsbp,
            tc.tile_pool(name="outp", bufs=4) as outp,
            tc.tile_pool(name="drp", bufs=4, space="DRAM") as drp,
        ):
            # ---- static loads (host pre-cast bf16; no on-chip casts) ----
            w_bf = {}
            for nm, d in (("wqt", wqt_d), ("wkt", wkt_d), ("wvt", wvt_d)):
                ld = singles.tile([128, CT, C], BF16, tag=f"{nm}_ld")
                nc.sync.dma_start(out=ld, in_=d.rearrange("(t p) o -> p t o", p=128))
                w_bf[nm] = ld
            wqt_sb, wkt_sb, wvt_sb = w_bf["wqt"], w_bf["wkt"], w_bf["wvt"]

            bqp_sb = singles.tile([128, CT], F32)
            nc.sync.dma_start(out=bqp_sb, in_=bqp_d[:, :])
            bop_sb = singles.tile([128, CT], F32)
            nc.sync.dma_start(out=bop_sb, in_=bop_d[:, :])

            x_bf = sbig.tile([128, CT, S], BF16, name="x_bf")
            nc.scalar.dma_start(out=x_bf, in_=xf_d.rearrange("(t p) s -> p t s", p=128))
            xq_bf = sbig.tile([128, CT, SQ], BF16, name="xq_bf")
            nc.sync.dma_start(out=xq_bf, in_=xqb_d.rearrange("(t p) s -> p t s", p=128))
            xq_ld = sbig.tile([128, CT, SQ], F32, name="xq_ld")
            nc.gpsimd.dma_start(out=xq_ld, in_=xq_d.rearrange("(t p) s -> p t s", p=128))

            wot_ld = singles.tile([128, CT, C], F32, tag="wot_ld")
            nc.sync.dma_start(out=wot_ld, in_=wot_d.rearrange("(t p) o -> p t o", p=128))
            wot_sb = singles.tile([128, CT, C], F32R, tag="wot_rb")
            nc.vector.tensor_copy(out=wot_sb, in_=wot_ld)

            k_t = [sbig.tile([128, S], BF16, name=f"k{t}") for t in range(CT)]
            q_t = [sbig.tile([128, SQ], BF16, name=f"q{t}") for t in range(CT)]
            # V^T with ones denominator column: [t, st, head, HD+1]
            vt = sbig.tile([128, NTT, NH, HD + 1], BF16, name="vt")
            nc.vector.memset(vt, 1.0)
            att_j = [sbig.tile([128, CT, ln], F32R, name=f"att{i}")
                     for i, (j0, ln) in enumerate(JCH)]

            den_d = drp.tile([NH, SQ], F32, tag="den")
            rec_d = drp.tile([NH, SQ], F32, tag="rec")

            # ---- phase A: projections (all-bf16 matmuls) ----------------
            # copies out of PSUM ride ScalarE (ACT), freeing VectorE for exp
            def k_proj(ot, psA):
                for (j0, ln) in KCH:
                    ps = psA.tile([128, 512], F32, tag="proj", name=f"kp{ot}{j0}")
                    for kt in range(CT):
                        nc.tensor.matmul(
                            ps[:, 0:ln],
                            lhsT=wkt_sb[:, kt, ot * 128:(ot + 1) * 128],
                            rhs=x_bf[:, kt, j0:j0 + ln],
                            start=(kt == 0), stop=(kt == CT - 1),
                        )
                    nc.scalar.activation(
                        out=k_t[ot][:, j0:j0 + ln], in_=ps[:, 0:ln], func=AF.Copy,
                    )

            def q_proj(ot, psA):
                for (j0, ln) in QCH:
                    ps = psA.tile([128, 512], F32, tag="proj", name=f"qp{ot}{j0}")
                    for kt in range(CT):
                        nc.tensor.matmul(
                            ps[:, 0:ln],
                            lhsT=wqt_sb[:, kt, ot * 128:(ot + 1) * 128],
                            rhs=xq_bf[:, kt, j0:j0 + ln],
                            start=(kt == 0), stop=(kt == CT - 1),
                        )
                    nc.scalar.activation(
                        out=q_t[ot][:, j0:j0 + ln], in_=ps[:, 0:ln],
                        func=AF.Identity, bias=bqp_sb[:, ot:ot + 1],
                    )

            def v_proj(sv, psA):
                ps = psA.tile([128, 2, C], F32, tag="proj", name=f"vp{sv}")
                for sti in range(2):
                    st = 2 * sv + sti
                    for kt in range(CT):
                        nc.tensor.matmul(
                            ps[:, sti, :],
                            lhsT=x_bf[:, kt, st * 128:(st + 1) * 128],
                            rhs=wvt_sb[:, kt, :],
                            start=(kt == 0), stop=(kt == CT - 1),
                        )
                nc.scalar.activation(
                    out=vt[:, 2 * sv:2 * sv + 2, :, 0:HD],
                    in_=ps.rearrange("p s (h d) -> p s h d", h=NH),
                    func=AF.Copy,
                )

            with tc.tile_pool(name="psA", bufs=4, space="PSUM") as psA:
                k_proj(0, psA)
                q_proj(0, psA)
                k_proj(1, psA)
                q_proj(1, psA)
                for sv in range(NTT // 2):
                    v_proj(sv, psA)

            if DEBUG:
                nc.sync.dma_start(out=dbg["k0"][:, :], in_=k_t[0])
                nc.sync.dma_start(out=dbg["q0"][:, :], in_=q_t[0])
                nc.sync.dma_start(out=dbg["vt"][:, :, :, :], in_=vt)

            # ---- phase B: chunk-major attention -------------------------
            # head h: channel tile ct=h//4, offset co=32*(h%4).
            # pair hp: heads (2hp, 2hp+1). av tile side a: pairs (2a, 2a+1),
            # PSUM layout [128, pair-slot s, 256]: rows 64*hi + [0:32] vals,
            # row 64*hi+32 denominator.
            exp_acc = [0.5]

            def emit_exp(sc, ex, ln):
                exp_acc[0] += A_SHARE
                if exp_acc[0] >= 1.0:
                    exp_acc[0] -= 1.0
                    nc.scalar.activation(
                        out=ex[:, :, 0:ln], in_=sc[:, :, 0:ln], func=AF.Exp,
                    )
                else:
                    nc.vector.tensor_scalar(
                        out=ex[:, :, 0:ln].bitcast(I16), in0=sc[:, :, 0:ln],
                        scalar1=EXP_A, scalar2=EXP_B,
                        op0=ALU.mult, op1=ALU.add,
                    )

            def attention_chunk(jidx, scp, avp):
                j0, ln = JCH[jidx]
                js = slice(j0, j0 + ln)
                # Each pair gets a FULL PSUM bank ([128, 512] f32): two
                # concurrently-open accumulations may not share a bank at
                # the same partitions (start=True clears the whole bank row
                # for the partitions written). The two hi regions inside a
                # pair are partition-disjoint (rows 0-32 / 64-96). The two
                # pair-duos run sequentially so only 2 av banks are live.
                avs_t = []
                for a in range(2):              # a: pair-duo = channel tile
                    av = [avp.tile([128, 512], F32, tag="av",
                                   name=f"av{jidx}{a}{hpi}")
                          for hpi in range(2)]
                    for g in range(NTG):
                        sc = [scp.tile([128, 2 * TG, 256], F32, tag="sc",
                                       name=f"sc{jidx}{g}{a}{hpi}")
                              for hpi in range(2)]
                        for tt in range(TG):
                            t0 = (g * TG + tt) * 128
                            for hpi in range(2):    # pair within duo
                                for hi in range(2):
                                    co = 32 * (2 * hpi + hi)
                                    nc.tensor.matmul(
                                        sc[hpi][:, hi * TG + tt, 0:ln],
                                        lhsT=k_t[a][co:co + HD, t0:t0 + 128],
                                        rhs=q_t[a][co:co + HD, js],
                                        start=True, stop=True,
                                        tile_position=(co, 0),
                                    )
                        ex = [expsp.tile([128, 2 * TG, 256], BF16, tag="ex",
                                         name=f"ex{jidx}{g}{a}{hpi}")
                              for hpi in range(2)]
                        for hpi in range(2):
                            emit_exp(sc[hpi], ex[hpi], ln)
                        if DEBUG and jidx == 0 and g == 0 and a == 0:
                            for hpi in range(2):
                                nc.sync.dma_start(out=dbg[f"ex{hpi}"][:, :, :],
                                                  in_=ex[hpi])
                        for tt in range(TG):
                            st = g * TG + tt
                            first = (g == 0 and tt == 0)
                            last = (g == NTG - 1 and tt == TG - 1)
                            for hpi in range(2):
                                for hi in range(2):
                                    h = 4 * a + 2 * hpi + hi
                                    nc.tensor.matmul(
                                        av[hpi][64 * hi:64 * hi + HD + 1, 0:ln],
                                        lhsT=vt[:, st, h, :],
                                        rhs=ex[hpi][:, hi * TG + tt, 0:ln],
                                        start=first, stop=last,
                                        tile_position=(0, 64 * hi),
                                        skip_group_check=True,
                                    )
                    # exit av from PSUM (ScalarE); denominators -> DRAM
                    for hpi in range(2):
                        avs = avsbp.tile([97, 256], F32, tag="avs",
                                         name=f"avs{jidx}{a}{hpi}")
                        avs_t.append(avs)
                        nc.scalar.activation(
                            out=avs[:, 0:ln], in_=av[hpi][0:97, 0:ln],
                            func=AF.Copy,
                        )
                        for hi in range(2):
                            h = 4 * a + 2 * hpi + hi
                            nc.sync.dma_start(
                                out=den_d[h:h + 1, j0:j0 + ln],
                                in_=avs[64 * hi + 32:64 * hi + 33, 0:ln],
                            )
                # batched reciprocal of all 8 denominators for this chunk
                nh = ln // 128
                dsb = dsbp.tile([128, NH, 4], F32, tag="dsb", name=f"dsb{jidx}")
                rcb = dsbp.tile([128, NH, 4], F32, tag="rcb", name=f"rcb{jidx}")
                for f in range(nh):
                    qf = j0 + 128 * f
                    nc.gpsimd.dma_start(
                        out=dsb[:, :, f:f + 1],
                        in_=den_d[:, qf:qf + 128].rearrange("h (p o) -> p h o", o=1),
                    )
                nc.vector.reciprocal(rcb[:, :, 0:nh], dsb[:, :, 0:nh])
                for f in range(nh):
                    qf = j0 + 128 * f
                    nc.gpsimd.dma_start(
                        out=rec_d[:, qf:qf + 128].rearrange("h (p o) -> p h o", o=1),
                        in_=rcb[:, :, f:f + 1],
                    )
                # broadcast recips + fused normalize multiply -> f32r
                for p in range(4):              # pair index; heads 2p, 2p+1
                    a, hpi = p // 2, p % 2
                    bc = bcp.tile([128, 256], F32, tag="bc", name=f"bc{jidx}{p}")
                    # rows 0-63 <- rec[2p], rows 64-127 <- rec[2p+1]
                    # NOT on the scalar queue: a DMA trigger waiting on the
                    # rec roundtrip would block the ACT engine's exp stream.
                    for hi in range(2):
                        rr = rec_d[2 * p + hi:2 * p + hi + 1, j0:j0 + ln]
                        nc.gpsimd.dma_start(
                            out=bc[64 * hi:64 * hi + 64, 0:ln],
                            in_=bass.AP(
                                tensor=rr.tensor, offset=rr.offset,
                                ap=[[0, 64]] + [list(x) for x in rr.ap[1:]],
                            ),
                        )
                    if DEBUG and jidx == 0 and p == 0:
                        nc.sync.dma_start(out=dbg["avs0"][:, :], in_=avs_t[0])
                        nc.sync.dma_start(out=dbg["bc0"][:, :], in_=bc)
                    nrm = avsbp.tile([97, 256], F32R, tag="nrm",
                                     name=f"nrm{jidx}{p}")
                    nc.vector.tensor_tensor(
                        out=nrm[:, 0:ln], in0=avs_t[p][:, 0:ln],
                        in1=bc[0:97, 0:ln], op=ALU.mult,
                    )
                    # att layout: head h -> ct h//4, rows 32*(h%4)
                    r0 = 64 * (p % 2)
                    nc.sync.dma_start(
                        out=att_j[jidx][r0:r0 + 32, a, 0:ln],
                        in_=nrm[0:32, 0:ln],
                    )
                    nc.sync.dma_start(
                        out=att_j[jidx][r0 + 32:r0 + 64, a, 0:ln],
                        in_=nrm[64:96, 0:ln],
                    )

            with (
                tc.tile_pool(name="scp", bufs=2, space="PSUM") as scp,
                tc.tile_pool(name="avp", bufs=2, space="PSUM") as avp,
            ):
                # chunk 0 at high priority so its scores/exp start the
                # moment the needed K/Q land, demoting the projection
                # backlog to PE gap-filler work.
                with tc.high_priority():
                    attention_chunk(0, scp, avp)
                for jidx in range(1, len(JCH)):
                    attention_chunk(jidx, scp, avp)

            if DEBUG:
                nc.sync.dma_start(out=dbg["den"][:, :], in_=den_d[:, :])
                nc.sync.dma_start(out=dbg["rec"][:, :], in_=rec_d[:, :])
                nc.sync.dma_start(out=dbg["att0"][:, :, :],
                                  in_=att_j[0][:, :, :].bitcast(F32))

            # ---- phase C: output projection + residual ------------------
            out_r = out_d.rearrange("(t p) q -> p t q", p=128)

            def out_proj_chunk(jidx, psC):
                j0, ln = JCH[jidx]
                js = slice(j0, j0 + ln)
                for ot in range(CT):
                    # full PSUM bank per accumulator: co-tenant accumulation
                    # groups in one bank wipe each other on start=True
                    ps = psC.tile([128, 512], F32, tag="cps", name=f"cps{jidx}{ot}")
                    for kt in range(CT):
                        nc.tensor.matmul(
                            ps[:, 0:ln],
                            lhsT=wot_sb[:, kt, ot * 128:(ot + 1) * 128],
                            rhs=att_j[jidx][:, kt, 0:ln],
                            start=(kt == 0), stop=(kt == CT - 1),
                        )
                    ob = outp.tile([128, 256], F32, tag="ob", name=f"ob{jidx}{ot}")
                    nc.vector.scalar_tensor_tensor(
                        out=ob[:, 0:ln], in0=ps[:, 0:ln],
                        scalar=bop_sb[:, ot:ot + 1], in1=xq_ld[:, ot, js],
                        op0=ALU.add, op1=ALU.add,
                    )
                    nc.sync.dma_start(out=out_r[:, ot, js], in_=ob[:, 0:ln])

            with tc.tile_pool(name="psC", bufs=4, space="PSUM") as psC:
                for jidx in range(len(JCH)):
                    out_proj_chunk(jidx, psC)
    return nc


_NC = None
LAST_RESULTS = None
TRACE = False


def _get_nc():
    global _NC
    if _NC is None:
        _NC = _build_nc()
    return _NC


def kernel(x, Wq, bq, Wk, bk, Wv, bv, Wo, bo):
    global LAST_RESULTS
    bf = ml_dtypes.bfloat16
    x = np.ascontiguousarray(np.asarray(x, dtype=np.float32).reshape(B, C, S))
    x_bf = x.astype(bf)
    Wq = np.asarray(Wq, dtype=np.float32)
    Wo = np.asarray(Wo, dtype=np.float32)
    bv = np.asarray(bv, dtype=np.float32)
    wqt = np.ascontiguousarray((Wq * SCALE).T.astype(bf))
    wkt = np.ascontiguousarray(np.asarray(Wk, dtype=np.float32).T.astype(bf))
    wvt = np.ascontiguousarray(np.asarray(Wv, dtype=np.float32).T.astype(bf))
    wot = np.ascontiguousarray(Wo.T)
    bqp = np.ascontiguousarray(
        (np.asarray(bq, dtype=np.float32) * SCALE).reshape(CT, 128).T)
    bo2 = np.asarray(bo, dtype=np.float32) + Wo @ bv
    bop = np.ascontiguousarray(bo2.reshape(CT, 128).T)

    in_maps = []
    for core in range(N_CORES):
        b, half = divmod(core, 2)
        qlo = half * SQ
        in_maps.append({
            "xfb": x_bf[b],
            "xqb": np.ascontiguousarray(x_bf[b][:, qlo:qlo + SQ]),
            "xq": np.ascontiguousarray(x[b][:, qlo:qlo + SQ]),
            "wqtb": wqt, "wktb": wkt, "wvtb": wvt, "wot": wot,
            "bqp": bqp, "bop": bop,
        })

    res = run_bass_kernel_spmd(_get_nc(), in_maps, list(range(N_CORES)), trace=TRACE)
    LAST_RESULTS = res

    out = np.empty((B, C, S), dtype=np.float32)
    for core in range(N_CORES):
        b, half = divmod(core, 2)
        qlo = half * SQ
        out[b][:, qlo:qlo + SQ] = res.results[core]["out"]
    return out.reshape(B, C, HH, WW)


# revision 30
# speedup vs baseline: 1.1053x; 1.1026x over previous
"""Multi-head self-attention (B=4, C=256, H=W=48, NH=8) on 8 TRN2 NeuronCores.

Sharding: 8 shards = 4 batches x 2 query-halves (no collectives).

v2 design (vs baseline at 252us):
  - exp split across ScalarE (true exp, ~55%) and VectorE (Schraudolph
    bit-trick exp: one tensor_scalar f32->int16 rint-convert bitcast over a
    bf16 tile; max rel err ~3.3%, washes out in softmax + residual).
  - chunk-major attention: all 4 head-pairs advance together per q-chunk,
    so consecutive score matmuls cycle PE row groups 0/32/64/96 (4-way
    tile_position packing) and the PE stream stays dense (HAM warm).
  - normalization: ScalarE exits av [0:97] PSUM->SBUF; denominators DMA to
    DRAM, ONE batched [128,16] DVE reciprocal per chunk, recip rows DMA
    back broadcast (stride-0 partition from DRAM), one fused [97,2,256]
    DVE multiply writes f32r; SBUF->SBUF DMAs shift rows into att layout.
    (replaces forty 1.6us single-partition RECIPROCALs)
  - host folds: V-bias into bo' = bo + Wo bv; softmax SCALE into Wq/bq;
    x/weights pre-cast to bf16 on host (no on-chip casts).
  - out-projection: fused (ps + bo')+x residual in one scalar_tensor_tensor.
"""

import numpy as np
import ml_dtypes

import concourse.bass as bass
import concourse.mybir as mybir
import concourse.tile as tile
from concourse.vector_clock import ScopedClock
from concourse.bass_utils import run_bass_kernel_spmd

B, C, HH, WW = 4, 256, 48, 48
S = HH * WW            # 2304
NH, HD = 8, 32
SCALE = HD ** -0.5
SQ = S // 2            # 1152 queries per core
CT = C // 128          # 2 channel tiles
NTT = S // 128         # 18 t-tiles
TG = 3                 # t-tiles per exp group
NTG = NTT // TG        # 6

F32 = mybir.dt.float32
F32R = mybir.dt.float32r
BF16 = mybir.dt.bfloat16
I16 = mybir.dt.int16
AF = mybir.ActivationFunctionType
ALU = mybir.AluOpType

N_CORES = 8

# attention/output q-chunks: 4x256 + 1x128
JCH = [(0, 256), (256, 256), (512, 256), (768, 256), (1024, 128)]
# projection chunks
KCH = [(0, 512), (512, 512), (1024, 512), (1536, 512), (2048, 256)]
QCH = [(0, 512), (512, 512), (1024, 128)]

# Schraudolph exp constants (scores arrive pre-scaled by SCALE via Wq/bq):
# bf16_bits(exp(x)) ~= rint(x * 128/ln2 + (127*128 - c))
EXP_A = 128.0 / float(np.log(2.0))
EXP_B = 127.0 * 128.0 - 0.0430357 * 128.0

# fraction of exp units sent to ScalarE (rest go to VectorE Schraudolph)
A_SHARE = 0.51

DEBUG = False


class _TileContextP(tile.TileContext):
    """TileContext adapted to a walrus that allows 1 sem wait/instruction.

    After Tile scheduling, every instruction carrying N>1 sem waits is
    rewritten to keep its last wait; the other N-1 waits move onto
    fresh single-wait nops inserted just before it on the same engine
    (engines execute their stream in order, so blocking at the nop is
    equivalent). The kernel-tail drain is built the same way.
    """

    def _split_multi_waits(self):
        nc = self.nc
        for fn in nc.m.functions:
            for bb in fn.blocks:
                new_insts = []
                for inst in bb.instructions:
                    si = inst.sync_info
                    if si is not None and len(si.on_wait) > 1:
                        waits = list(si.on_wait)
                        for w in waits[:-1]:
                            nop = mybir.InstNoOp(
                                name=nc.get_next_instruction_name(),
                                engine=inst.engine,
                                ins=[], outs=[],
                                sync_info=mybir.SyncInfo(on_wait=[w], on_update=[]),
                                bass_nofuse=True,
                            )
                            nc.register_instruction(nop, overwrite=True)
                            new_insts.append(nop)
                        inst.sync_info = mybir.SyncInfo(
                            on_wait=[waits[-1]], on_update=list(si.on_update)
                        )
                    new_insts.append(inst)
                bb.instructions = new_insts

    def _drain_and_barrier(self, tick_clock, wait_clock):
        carrier = self.nc.sync.nop(nofuse=True)
        wait_clock.add_sem_waits(
            carrier.ins, ScopedClock({None: tick_clock.global_clock})
        )
        self.nc.sync.drain()
        self.nc.all_engine_barrier()
        assert self.sems is not None
        popped = self.nc._tile_sem_poison_stack.pop()
        assert popped is self._sem_poison
        self.nc.clear_and_free_semaphores(list(self.sems.allocated().values()))
        self.nc.all_engine_barrier()
        self._split_multi_waits()


def _build_nc():
    nc = bass.Bass()

    xf_d = nc.dram_tensor("xfb", [C, S], BF16, kind="ExternalInput")
    xqb_d = nc.dram_tensor("xqb", [C, SQ], BF16, kind="ExternalInput")
    xq_d = nc.dram_tensor("xq", [C, SQ], F32, kind="ExternalInput")
    wqt_d = nc.dram_tensor("wqtb", [C, C], BF16, kind="ExternalInput")
    wkt_d = nc.dram_tensor("wktb", [C, C], BF16, kind="ExternalInput")
    wvt_d = nc.dram_tensor("wvtb", [C, C], BF16, kind="ExternalInput")
    wot_d = nc.dram_tensor("wot", [C, C], F32, kind="ExternalInput")
    bqp_d = nc.dram_tensor("bqp", [128, CT], F32, kind="ExternalInput")
    bop_d = nc.dram_tensor("bop", [128, CT], F32, kind="ExternalInput")
    out_d = nc.dram_tensor("out", [C, SQ], F32, kind="ExternalOutput")

    dbg = {}
    if DEBUG:
        dbg["k0"] = nc.dram_tensor("dbg_k0", [128, S], BF16, kind="ExternalOutput")
        dbg["q0"] = nc.dram_tensor("dbg_q0", [128, SQ], BF16, kind="ExternalOutput")
        dbg["vt"] = nc.dram_tensor("dbg_vt", [128, NTT, NH, HD + 1], BF16,
                                   kind="ExternalOutput")
        dbg["ex0"] = nc.dram_tensor("dbg_ex0", [128, 2 * TG, 256], BF16,
                                    kind="ExternalOutput")
        dbg["ex1"] = nc.dram_tensor("dbg_ex1", [128, 2 * TG, 256], BF16,
                                    kind="ExternalOutput")
        dbg["avs0"] = nc.dram_tensor("dbg_avs0", [97, 256], F32,
                                     kind="ExternalOutput")
        dbg["den"] = nc.dram_tensor("dbg_den", [NH, SQ], F32, kind="ExternalOutput")
        dbg["rec"] = nc.dram_tensor("dbg_rec", [NH, SQ], F32, kind="ExternalOutput")
        dbg["bc0"] = nc.dram_tensor("dbg_bc0", [128, 256], F32,
                                    kind="ExternalOutput")
        dbg["att0"] = nc.dram_tensor("dbg_att0", [128, CT, 256], F32,
                                     kind="ExternalOutput")

    with _TileContextP(nc) as tc:
        with (
            tc.tile_pool(name="singles", bufs=1) as singles,
            tc.tile_pool(name="sbig", bufs=1) as sbig,
            tc.tile_pool(name="expsp", bufs=8) as expsp,
            tc.tile_pool(name="avsbp", bufs=8) as avsbp,
            tc.tile_pool(name="nrmp", bufs=8) as nrmp,
            tc.tile_pool(name="bcp", bufs=8) as bcp,
            tc.tile_pool(name="dsbp", bufs=4) as dsbp,
            tc.tile_pool(name="outp", bufs=4) as outp,
            tc.tile_pool(name="drp", bufs=10, space="DRAM") as drp,
        ):
            # ---- static loads (host pre-cast bf16; no on-chip casts) ----
            w_bf = {}
            for nm, d in (("wqt", wqt_d), ("wkt", wkt_d), ("wvt", wvt_d)):
                ld = singles.tile([128, CT, C], BF16, tag=f"{nm}_ld")
                nc.sync.dma_start(out=ld, in_=d.rearrange("(t p) o -> p t o", p=128))
                w_bf[nm] = ld
            wqt_sb, wkt_sb, wvt_sb = w_bf["wqt"], w_bf["wkt"], w_bf["wvt"]

            bqp_sb = singles.tile([128, CT], F32)
            nc.sync.dma_start(out=bqp_sb, in_=bqp_d[:, :])
            bop_sb = singles.tile([128, CT], F32)
            nc.sync.dma_start(out=bop_sb, in_=bop_d[:, :])

            # chunked input loads across all three DMA queues so the first
            # projections can start within a few us
            x_bf = sbig.tile([128, CT, S], BF16, name="x_bf")
            xr = xf_d.rearrange("(t p) s -> p t s", p=128)
            qs_ = [nc.sync, nc.scalar, nc.gpsimd]
            for i, (j0, ln) in enumerate(KCH):
                qs_[i % 3].dma_start(out=x_bf[:, :, j0:j0 + ln],
                                     in_=xr[:, :, j0:j0 + ln])
            xq_bf = sbig.tile([128, CT, SQ], BF16, name="xq_bf")
            xqr = xqb_d.rearrange("(t p) s -> p t s", p=128)
            for i, (j0, ln) in enumerate(QCH):
                qs_[(i + 2) % 3].dma_start(out=xq_bf[:, :, j0:j0 + ln],
                                           in_=xqr[:, :, j0:j0 + ln])
            xq_ld = sbig.tile([128, CT, SQ], F32, name="xq_ld")
            nc.gpsimd.dma_start(out=xq_ld, in_=xq_d.rearrange("(t p) s -> p t s", p=128))

            wot_ld = singles.tile([128, CT, C], F32, tag="wot_ld")
            nc.sync.dma_start(out=wot_ld, in_=wot_d.rearrange("(t p) o -> p t o", p=128))
            wot_sb = singles.tile([128, CT, C], F32R, tag="wot_rb")
            nc.vector.tensor_copy(out=wot_sb, in_=wot_ld)

            k_t = [sbig.tile([128, S], BF16, name=f"k{t}") for t in range(CT)]
            q_t = [sbig.tile([128, SQ], BF16, name=f"q{t}") for t in range(CT)]
            # V^T with ones denominator column: [t, st, head, HD+1]
            vt = sbig.tile([128, NTT, NH, HD + 1], BF16, name="vt")
            nc.vector.memset(vt, 1.0)
            att_j = [sbig.tile([128, CT, ln], F32R, name=f"att{i}")
                     for i, (j0, ln) in enumerate(JCH)]


            # ---- phase A: projections (all-bf16 matmuls) ----------------
            # copies out of PSUM ride ScalarE (ACT), freeing VectorE for exp
            def k_proj(ot, psA):
                for (j0, ln) in KCH:
                    ps = psA.tile([128, 512], F32, tag="proj", name=f"kp{ot}{j0}")
                    for kt in range(CT):
                        nc.tensor.matmul(
                            ps[:, 0:ln],
                            lhsT=wkt_sb[:, kt, ot * 128:(ot + 1) * 128],
                            rhs=x_bf[:, kt, j0:j0 + ln],
                            start=(kt == 0), stop=(kt == CT - 1),
                        )
                    nc.scalar.activation(
                        out=k_t[ot][:, j0:j0 + ln], in_=ps[:, 0:ln], func=AF.Copy,
                    )

            def q_proj(ot, psA):
                for (j0, ln) in QCH:
                    ps = psA.tile([128, 512], F32, tag="proj", name=f"qp{ot}{j0}")
                    for kt in range(CT):
                        nc.tensor.matmul(
                            ps[:, 0:ln],
                            lhsT=wqt_sb[:, kt, ot * 128:(ot + 1) * 128],
                            rhs=xq_bf[:, kt, j0:j0 + ln],
                            start=(kt == 0), stop=(kt == CT - 1),
                        )
                    nc.scalar.activation(
                        out=q_t[ot][:, j0:j0 + ln], in_=ps[:, 0:ln],
                        func=AF.Identity, bias=bqp_sb[:, ot:ot + 1],
                    )

            def v_proj(sv, psA):
                ps = psA.tile([128, 2, C], F32, tag="proj", name=f"vp{sv}")
                for sti in range(2):
                    st = 2 * sv + sti
                    for kt in range(CT):
                        nc.tensor.matmul(
                            ps[:, sti, :],
                            lhsT=x_bf[:, kt, st * 128:(st + 1) * 128],
                            rhs=wvt_sb[:, kt, :],
                            start=(kt == 0), stop=(kt == CT - 1),
                        )
                nc.scalar.activation(
                    out=vt[:, 2 * sv:2 * sv + 2, :, 0:HD],
                    in_=ps.rearrange("p s (h d) -> p s h d", h=NH),
                    func=AF.Copy,
                )

            with tc.tile_pool(name="psA", bufs=4, space="PSUM") as psA:
                k_proj(0, psA)
                q_proj(0, psA)
                k_proj(1, psA)
                q_proj(1, psA)
                for sv in range(NTT // 2):
                    v_proj(sv, psA)

            if DEBUG:
                nc.sync.dma_start(out=dbg["k0"][:, :], in_=k_t[0])
                nc.sync.dma_start(out=dbg["q0"][:, :], in_=q_t[0])
                nc.sync.dma_start(out=dbg["vt"][:, :, :, :], in_=vt)

            # ---- phase B: chunk-major attention -------------------------
            # head h: channel tile ct=h//4, offset co=32*(h%4).
            # pair hp: heads (2hp, 2hp+1). av tile side a: pairs (2a, 2a+1),
            # PSUM layout [128, pair-slot s, 256]: rows 64*hi + [0:32] vals,
            # row 64*hi+32 denominator.
            exp_acc = [0.5]

            def emit_exp(sc, ex, ln):
                exp_acc[0] += A_SHARE
                if exp_acc[0] >= 1.0:
                    exp_acc[0] -= 1.0
                    nc.scalar.activation(
                        out=ex[:, :, 0:ln], in_=sc[:, :, 0:ln], func=AF.Exp,
                    )
                else:
                    nc.vector.tensor_scalar(
                        out=ex[:, :, 0:ln].bitcast(I16), in0=sc[:, :, 0:ln],
                        scalar1=EXP_A, scalar2=EXP_B,
                        op0=ALU.mult, op1=ALU.add,
                    )

            def attention_chunk(jidx, scp, avp):
                j0, ln = JCH[jidx]
                js = slice(j0, j0 + ln)
                dden = drp.tile([NH, 256], F32, tag="den", name=f"den{jidx}")
                drec = drp.tile([NH, 256], F32, tag="rec", name=f"rec{jidx}")
                # Each pair gets a FULL PSUM bank ([128, 512] f32): two
                # concurrently-open accumulations may not share a bank at
                # the same partitions (start=True clears the whole bank row
                # for the partitions written). The two hi regions inside a
                # pair are partition-disjoint (rows 0-32 / 64-96). The two
                # pair-duos run sequentially so only 2 av banks are live.
                avs_t = []
                for a in range(2):              # a: pair-duo = channel tile
                    av = [avp.tile([128, 512], F32, tag="av",
                                   name=f"av{jidx}{a}{hpi}")
                          for hpi in range(2)]
                    for g in range(NTG):
                        sc = [scp.tile([128, 2 * TG, 256], F32, tag="sc",
                                       name=f"sc{jidx}{g}{a}{hpi}")
                              for hpi in range(2)]
                        for tt in range(TG):
                            t0 = (g * TG + tt) * 128
                            for hpi in range(2):    # pair within duo
                                for hi in range(2):
                                    co = 32 * (2 * hpi + hi)
                                    nc.tensor.matmul(
                                        sc[hpi][:, hi * TG + tt, 0:ln],
                                        lhsT=k_t[a][co:co + HD, t0:t0 + 128],
                                        rhs=q_t[a][co:co + HD, js],
                                        start=True, stop=True,
                                        tile_position=(co, 0),
                                    )
                        ex = [expsp.tile([128, 2 * TG, 256], BF16, tag="ex",
                                         name=f"ex{jidx}{g}{a}{hpi}")
                              for hpi in range(2)]
                        for hpi in range(2):
                            emit_exp(sc[hpi], ex[hpi], ln)
                        if DEBUG and jidx == 0 and g == 0 and a == 0:
                            for hpi in range(2):
                                nc.sync.dma_start(out=dbg[f"ex{hpi}"][:, :, :],
                                                  in_=ex[hpi])
                        for tt in range(TG):
                            st = g * TG + tt
                            first = (g == 0 and tt == 0)
                            last = (g == NTG - 1 and tt == TG - 1)
                            for hpi in range(2):
                                for hi in range(2):
                                    h = 4 * a + 2 * hpi + hi
                                    nc.tensor.matmul(
                                        av[hpi][64 * hi:64 * hi + HD + 1, 0:ln],
                                        lhsT=vt[:, st, h, :],
                                        rhs=ex[hpi][:, hi * TG + tt, 0:ln],
                                        start=first, stop=last,
                                        tile_position=(0, 64 * hi),
                                        skip_group_check=True,
                                    )
                    # exit av from PSUM (ScalarE); denominators -> DRAM
                    for hpi in range(2):
                        avs = avsbp.tile([97, 256], F32, tag="avs",
                                         name=f"avs{jidx}{a}{hpi}")
                        avs_t.append(avs)
                        nc.scalar.activation(
                            out=avs[:, 0:ln], in_=av[hpi][0:97, 0:ln],
                            func=AF.Copy,
                        )
                        for hi in range(2):
                            h = 4 * a + 2 * hpi + hi
                            nc.sync.dma_start(
                                out=dden[h:h + 1, 0:ln],
                                in_=avs[64 * hi + 32:64 * hi + 33, 0:ln],
                            )
                # batched reciprocal of all 8 denominators for this chunk
                nh = ln // 128
                dsb = dsbp.tile([128, NH, 4], F32, tag="dsb", name=f"dsb{jidx}")
                rcb = dsbp.tile([128, NH, 4], F32, tag="rcb", name=f"rcb{jidx}")
                for f in range(nh):
                    qf = 128 * f
                    nc.gpsimd.dma_start(
                        out=dsb[:, :, f:f + 1],
                        in_=dden[:, qf:qf + 128].rearrange("h (p o) -> p h o", o=1),
                    )
                nc.vector.reciprocal(rcb[:, :, 0:nh], dsb[:, :, 0:nh])
                for f in range(nh):
                    qf = 128 * f
                    nc.gpsimd.dma_start(
                        out=drec[:, qf:qf + 128].rearrange("h (p o) -> p h o", o=1),
                        in_=rcb[:, :, f:f + 1],
                    )
                # broadcast recips + fused normalize multiply -> f32r
                for p in range(4):              # pair index; heads 2p, 2p+1
                    a, hpi = p // 2, p % 2
                    bc = bcp.tile([128, 256], F32, tag="bc", name=f"bc{jidx}{p}")
                    # rows 0-63 <- rec[2p], rows 64-127 <- rec[2p+1]
                    # NOT on the scalar queue: a DMA trigger waiting on the
                    # rec roundtrip would block the ACT engine's exp stream.
                    for hi in range(2):
                        rr = drec[2 * p + hi:2 * p + hi + 1, 0:ln]
                        nc.gpsimd.dma_start(
                            out=bc[64 * hi:64 * hi + 64, 0:ln],
                            in_=bass.AP(
                                tensor=rr.tensor, offset=rr.offset,
                                ap=[[0, 64]] + [list(x) for x in rr.ap[1:]],
                            ),
                        )
                    if DEBUG and jidx == 0 and p == 0:
                        nc.sync.dma_start(out=dbg["avs0"][:, :], in_=avs_t[0])
                        nc.sync.dma_start(out=dbg["bc0"][:, :], in_=bc)
                    nrm = nrmp.tile([97, 256], F32R, tag="nrm",
                                     name=f"nrm{jidx}{p}")
                    nc.vector.tensor_tensor(
                        out=nrm[:, 0:ln], in0=avs_t[p][:, 0:ln],
                        in1=bc[0:97, 0:ln], op=ALU.mult,
                    )
                    # att layout: head h -> ct h//4, rows 32*(h%4)
                    r0 = 64 * (p % 2)
                    nc.sync.dma_start(
                        out=att_j[jidx][r0:r0 + 32, a, 0:ln],
                        in_=nrm[0:32, 0:ln],
                    )
                    nc.sync.dma_start(
                        out=att_j[jidx][r0 + 32:r0 + 64, a, 0:ln],
                        in_=nrm[64:96, 0:ln],
                    )

            with (
                tc.tile_pool(name="scp", bufs=2, space="PSUM") as scp,
                tc.tile_pool(name="avp", bufs=2, space="PSUM") as avp,
            ):
                # chunk 0 at high priority so its scores/exp start the
                # moment the needed K/Q land, demoting the projection
                # backlog to PE gap-filler work.
                with tc.high_priority():
                    attention_chunk(0, scp, avp)
                for jidx in range(1, len(JCH)):
                    attention_chunk(jidx, scp, avp)

            if DEBUG:
                nc.sync.dma_start(out=dbg["att0"][:, :, :],
                                  in_=att_j[0][:, :, :].bitcast(F32))

            # ---- phase C: output projection + residual ------------------
            out_r = out_d.rearrange("(t p) q -> p t q", p=128)

            def out_proj_chunk(jidx, psC):
                j0, ln = JCH[jidx]
                js = slice(j0, j0 + ln)
                for ot in range(CT):
                    # full PSUM bank per accumulator: co-tenant accumulation
                    # groups in one bank wipe each other on start=True
                    ps = psC.tile([128, 512], F32, tag="cps", name=f"cps{jidx}{ot}")
                    for kt in range(CT):
                        nc.tensor.matmul(
                            ps[:, 0:ln],
                            lhsT=wot_sb[:, kt, ot * 128:(ot + 1) * 128],
                            rhs=att_j[jidx][:, kt, 0:ln],
                            start=(kt == 0), stop=(kt == CT - 1),
                        )
                    ob = outp.tile([128, 256], F32, tag="ob", name=f"ob{jidx}{ot}")
                    nc.vector.scalar_tensor_tensor(
                        out=ob[:, 0:ln], in0=ps[:, 0:ln],
                        scalar=bop_sb[:, ot:ot + 1], in1=xq_ld[:, ot, js],
                        op0=ALU.add, op1=ALU.add,
                    )
                    nc.sync.dma_start(out=out_r[:, ot, js], in_=ob[:, 0:ln])

            with tc.tile_pool(name="psC", bufs=4, space="PSUM") as psC:
                for jidx in range(len(JCH)):
                    out_proj_chunk(jidx, psC)
    return nc


_NC = None
LAST_RESULTS = None
TRACE = False


def _get_nc():
    global _NC
    if _NC is None:
        _NC = _build_nc()
    return _NC


def kernel(x, Wq, bq, Wk, bk, Wv, bv, Wo, bo):
    global LAST_RESULTS
    bf = ml_dtypes.bfloat16
    x = np.ascontiguousarray(np.asarray(x, dtype=np.float32).reshape(B, C, S))
    x_bf = x.astype(bf)
    Wq = np.asarray(Wq, dtype=np.float32)
    Wo = np.asarray(Wo, dtype=np.float32)
    bv = np.asarray(bv, dtype=np.float32)
    wqt = np.ascontiguousarray((Wq * SCALE).T.astype(bf))
    wkt = np.ascontiguousarray(np.asarray(Wk, dtype=np.float32).T.astype(bf))
    wvt = np.ascontiguousarray(np.asarray(Wv, dtype=np.float32).T.astype(bf))
    wot = np.ascontiguousarray(Wo.T)
    bqp = np.ascontiguousarray(
        (np.asarray(bq, dtype=np.float32) * SCALE).reshape(CT, 128).T)
    bo2 = np.asarray(bo, dtype=np.float32) + Wo @ bv
    bop = np.ascontiguousarray(bo2.reshape(CT, 128).T)

    in_maps = []
    for core in range(N_CORES):
        b, half = divmod(core, 2)
        qlo = half * SQ
        in_maps.append({
            "xfb": x_bf[b],
            "xqb": np.ascontiguousarray(x_bf[b][:, qlo:qlo + SQ]),
            "xq": np.ascontiguousarray(x[b][:, qlo:qlo + SQ]),
            "wqtb": wqt, "wktb": wkt, "wvtb": wvt, "wot": wot,
            "bqp": bqp, "bop": bop,
        })

    res = run_bass_kernel_spmd(_get_nc(), in_maps, list(range(N_CORES)), trace=TRACE)
    LAST_RESULTS = res

    out = np.empty((B, C, S), dtype=np.float32)
    for core in range(N_CORES):
        b, half = divmod(core, 2)
        qlo = half * SQ
        out[b][:, qlo:qlo + SQ] = res.results[core]["out"]
    return out.reshape(B, C, HH, WW)
